# revision 19
# baseline (speedup 1.0000x reference)
"""GQA attention (B=4,S=1024,D=2048,H=32,KVH=8,HD=64) + RoPE, tensor-parallel
over the 8 kv-head groups across 8 NeuronCores.

v2 (trace-driven rewrite of the baseline):
  - score tiles double-buffered per head (f32 [128,1024], 4 banks) so the
    exp chain never blocks the next kb's score matmuls.
  - out-projection packs head PAIRS: avn2 [128,S] holds two heads' dims
    stacked, Wo pair rows stacked -> K=128 matmuls (256 instead of 512).
    Head-odd normalized output is moved to partitions 64:128 via a small
    SBUF->SBUF partition-shift DMA.
  - softmax reciprocals batched per (pair,qh): one Ln + one Exp on
    [1,1024] instead of per-head passes; broadcast via gpsimd
    partition_broadcast (no DRAM bounce).
  - y PSUM->SBUF copies alternate DVE/ACT to balance engine load.

Per-core pipeline (all-transposed layouts, no softmax-max pass):
  qT/kT/vT = W.T @ xT         (PE, bf16, D-contraction in 16 chunks of 128)
  RoPE: qrot = (q*cos) + A@(q*sin)   (rotate-half folded into a PE matmul A)
  sT = k_ropeT.T-block @ q_ropeT     ([keys,q] scores, K=64 contraction)
  pT = exp(sT/8)                     (ACT, no max subtraction; scores small)
  avT_aug = [v|1].T @ pT             (PE; row 64 = softmax denominator)
  avn = avT * bcast(1/denom)         (DVE; recip via exp(-ln) on ACT)
  y_partial = avn_pair.T @ Wo_pair   (PE, K=128), DMA out fp32; host sums.
"""

import numpy as np
import ml_dtypes

import concourse.bass as bass
import concourse.mybir as mybir
import concourse.tile as tile
from concourse import bacc
from concourse import bass_utils

BF16 = mybir.dt.bfloat16
F32 = mybir.dt.float32
BF = ml_dtypes.bfloat16

B, S, D = 4, 1024, 2048
H, KVH, HD = 32, 8, 64
NREP = H // KVH          # 4 q heads per core
T = B * S                # 4096 tokens
NC = 8                   # cores
QD = NREP * HD           # 256 q dims per core
KC = D // 128            # 16 contraction chunks
TB = 512                 # proj token-block
NTB = T // TB            # 8
AF = mybir.ActivationFunctionType

_CACHE = {}


def _build():
    key = "nc"
    if key in _CACHE:
        return _CACHE[key]
    nc = bacc.Bacc("TRN2", target_bir_lowering=False)
    # Pin all ACT table lookups to set 6 (natural_log_exp_and_others: has
    # Exp, Ln, Copy) so the kernel needs exactly one table load.
    import concourse.bacc as _bacc_mod
    _orig_tables = _bacc_mod.get_activation_tables

    def _pinned_tables(arch):
        items = list(_orig_tables(arch).items())
        return {k: (v if i == 6 else set()) for i, (k, v) in enumerate(items)}

    _bacc_mod.get_activation_tables = _pinned_tables

    xT_d = nc.dram_tensor("xT", (D, T), BF16, kind="ExternalInput")
    wq_d = nc.dram_tensor("wq", (D, QD), BF16, kind="ExternalInput")
    wkv_d = nc.dram_tensor("wkv", (D, 128), BF16, kind="ExternalInput")
    wo_d = nc.dram_tensor("wo", (QD, D), BF16, kind="ExternalInput")
    cos_d = nc.dram_tensor("cos2", (128, S), F32, kind="ExternalInput")
    sin_d = nc.dram_tensor("sin2", (128, S), F32, kind="ExternalInput")
    arot_d = nc.dram_tensor("arot", (128, 128), BF16, kind="ExternalInput")
    eye_d = nc.dram_tensor("eye64", (64, 64), BF16, kind="ExternalInput")
    y_d = nc.dram_tensor("y", (T, D), F32, kind="ExternalOutput")

    with tile.TileContext(nc) as tc:
        with (
            tc.tile_pool(name="const", bufs=1) as cpool,
            tc.tile_pool(name="persist", bufs=1) as ppool,
        ):
            # ---- constants ----
            wq_sb = cpool.tile([128, KC * QD], BF16, tag="wq")
            wq_dv = wq_d[:].rearrange("(c p) m -> p c m", p=128)
            wq_sv = wq_sb[:].rearrange("p (c m) -> p c m", c=KC)
            nc.sync.dma_start(out=wq_sv[:, 0:4, :], in_=wq_dv[:, 0:4, :])
            wkv_sb = cpool.tile([128, KC * 128], BF16, tag="wkv")
            nc.sync.dma_start(
                out=wkv_sb[:].rearrange("p (c m) -> p c m", c=KC),
                in_=wkv_d[:].rearrange("(c p) m -> p c m", p=128),
            )
            nc.sync.dma_start(out=wq_sv[:, 4:KC, :], in_=wq_dv[:, 4:KC, :])
            # Wo pair-stacked: pair p rows = Wo rows for heads 2p,2p+1
            wo2_sb = cpool.tile([128, 2 * D], BF16, tag="wo2")
            nc.sync.dma_start(out=wo2_sb[:, 0:D], in_=wo_d[0:128, :])
            nc.sync.dma_start(out=wo2_sb[:, D:2 * D], in_=wo_d[128:256, :])
            cos_sb = cpool.tile([128, S], F32, tag="cos")
            nc.sync.dma_start(out=cos_sb[:], in_=cos_d[:])
            sin_sb = cpool.tile([128, S], F32, tag="sin")
            nc.sync.dma_start(out=sin_sb[:], in_=sin_d[:])
            arot_sb = cpool.tile([128, 128], BF16, tag="arot")
            nc.sync.dma_start(out=arot_sb[:], in_=arot_d[:])
            eye_sb = cpool.tile([64, 64], BF16, tag="eye")
            nc.sync.dma_start(out=eye_sb[:], in_=eye_d[:])

            # ---- persistent activations ----
            qrope = [ppool.tile([128, T], BF16, tag=f"qrope{p}", name=f"qrope{p}")
                     for p in range(2)]
            kT2 = ppool.tile([128, T], BF16, tag="kT2")
            vtmpT = ppool.tile([64, T], BF16, tag="vtmpT")
            v_aug = [ppool.tile([128, 8 * 65], BF16, tag=f"vaug{b}", name=f"vaug{b}")
                     for b in range(B)]

            # ================= projection phase =================
            with (
                tc.tile_pool(name="xin", bufs=2) as xpool,
                tc.tile_pool(name="rtmp", bufs=3) as rpool,
                tc.tile_pool(name="pj", bufs=1, space="PSUM") as pj,
                tc.tile_pool(name="pshift", bufs=2, space="PSUM") as psh,
                tc.tile_pool(name="pvtr", bufs=2, space="PSUM") as pvt,
            ):
                for tb in range(NTB):
                    b, scol = tb // 2, (tb % 2) * TB
                    tcols = bass.ts(tb, TB)
                    xts = xpool.tile([128, KC * TB], BF16, tag="xts")
                    nc.sync.dma_start(
                        out=xts[:].rearrange("p (c n) -> p c n", c=KC),
                        in_=xT_d[:, tcols].rearrange("(c p) n -> p c n", p=128),
                    )
                    q0ps = pj.tile([128, TB], F32, tag="q0")
                    q1ps = pj.tile([128, TB], F32, tag="q1")
                    kvps = pj.tile([128, TB], F32, tag="kv")
                    for c in range(KC):
                        xc = xts[:, bass.ts(c, TB)]
                        st = dict(start=(c == 0), stop=(c == KC - 1))
                        nc.tensor.matmul(q0ps[:], wq_sb[:, c * QD:c * QD + 128], xc, **st)
                        nc.tensor.matmul(q1ps[:], wq_sb[:, c * QD + 128:(c + 1) * QD], xc, **st)
                        nc.tensor.matmul(kvps[:], wkv_sb[:, bass.ts(c, 128)], xc, **st)
                    css, sns = cos_sb[:, scol:scol + TB], sin_sb[:, scol:scol + TB]
                    # q pairs RoPE
                    for p, qps in ((0, q0ps), (1, q1ps)):
                        qsin = rpool.tile([128, TB], BF16, tag="qsin")
                        nc.vector.tensor_mul(qsin[:], qps[:], sns)
                        t1 = rpool.tile([128, TB], F32, tag="t1")
                        nc.vector.tensor_mul(t1[:], qps[:], css)
                        shift = psh.tile([128, TB], F32, tag="shift")
                        nc.tensor.matmul(shift[:], arot_sb[:], qsin[:], start=True, stop=True)
                        nc.vector.tensor_add(qrope[p][:, tcols], t1[:], shift[:])
                    # k RoPE on rows 0:64
                    ksin = rpool.tile([64, TB], BF16, tag="qsin")
                    nc.vector.tensor_mul(ksin[:], kvps[0:64, :], sns[0:64])
                    t1k = rpool.tile([64, TB], F32, tag="t1")
                    nc.vector.tensor_mul(t1k[:], kvps[0:64, :], css[0:64])
                    shk = psh.tile([128, TB], F32, tag="shift")
                    nc.tensor.matmul(shk[0:64, :], arot_sb[0:64, 0:64], ksin[:], start=True, stop=True)
                    nc.vector.tensor_add(kT2[0:64, tcols], t1k[:], shk[0:64, :])
                    # v: copy to a scratch at rows 64:128, DMA down to 0:64
                    vhi = rpool.tile([128, TB], BF16, tag="vhi")
                    nc.scalar.copy(vhi[64:128, :], kvps[64:128, :])
                    nc.sync.dma_start(out=vtmpT[:, tcols], in_=vhi[64:128, :])
                    if tb % 2 == 1:
                        # batch b complete: build v natural (+ones col)
                        for kb in range(8):
                            vtr = pvt.tile([128, 64], BF16, tag="vtr")
                            nc.tensor.transpose(
                                vtr[:], vtmpT[:, b * S + kb * 128:b * S + (kb + 1) * 128],
                                eye_sb[:],
                            )
                            nc.scalar.copy(v_aug[b][:, kb * 65:kb * 65 + 64], vtr[:])
                        nc.vector.memset(
                            v_aug[b][:].rearrange("p (k o) -> p k o", k=8)[:, :, 64:65], 1.0
                        )
                # duplicate k_rope to rows 64:128 (head-odd scores alignment)
                if tb == NTB - 1:
                    nc.sync.dma_start(out=kT2[64:128, :], in_=kT2[0:64, :])

            # ================= attention + output phase =================
            with (
                tc.tile_pool(name="prob", bufs=2) as prpool,
                tc.tile_pool(name="nrm", bufs=1) as npool,
                tc.tile_pool(name="rbc", bufs=2) as rbcpool,
                tc.tile_pool(name="scr", bufs=2) as scrpool,
                tc.tile_pool(name="avn2", bufs=2) as apool,
                tc.tile_pool(name="yout", bufs=2) as ypool,
                tc.tile_pool(name="psT", bufs=1, space="PSUM") as pst,
                tc.tile_pool(name="pav", bufs=1, space="PSUM") as pav,
                tc.tile_pool(name="py", bufs=2, space="PSUM") as py,
                tc.tile_pool(name="dscr", bufs=2, space="DRAM") as dpool,
            ):
                # ===== software-pipelined conveyor =====
                # Iteration ui, ticks k=0..7. Per tick emit:
                #   scores+exp(units[ui], kb=k)          <- feeds ACT
                #   AV chunks + qh0 recip of units[ui-1] <- PE filler
                #   qh1 recip tail of units[ui-2]
                #   out-proj half-blocks on a global tick schedule
                # so neither PE nor ACT ever sits behind a long foreign block.
                units = [(b, pr) for b in range(B) for pr in range(2)]
                NU = len(units)
                denb = ppool.tile([65, 2048], F32, tag="denb")
                rrow = ppool.tile([65, 2048], F32, tag="rrow")
                ustate = [dict() for _ in range(NU)]
                OUT_START = 27   # global tick when out-proj stream begins

                def scores_exp(st, kb):
                    b, pr = st["u"]
                    bS = b * S
                    kcols = slice(bS + kb * 128, bS + (kb + 1) * 128)
                    ctx = tc.high_priority()
                    ctx.__enter__()
                    for hh in range(2):
                        r0 = hh * 64
                        sps = pst.tile([128, 1024], F32, tag=f"sT{hh}",
                                       name=f"sps{hh}")
                        for qh in range(2):
                            nc.tensor.matmul(
                                sps[:, qh * 512:(qh + 1) * 512],
                                kT2[r0:r0 + 64, kcols],
                                qrope[pr][r0:r0 + 64,
                                          bS + qh * 512:bS + (qh + 1) * 512],
                                start=True, stop=True,
                            )
                        nc.scalar.activation(
                            st["probs"][hh][:, kb * S:(kb + 1) * S],
                            sps[:], AF.Exp, scale=0.125,
                        )
                    ctx.__exit__(None, None, None)

                def av_half(st, qh, hh, half):
                    b, _ = st["u"]
                    key = f"av{qh}{hh}"
                    if half == 0:
                        st[key] = pav.tile([65, 512], F32, tag=f"av{hh}",
                                           name=key)
                    avp = st[key]
                    for kb in range(half * 4, half * 4 + 4):
                        nc.tensor.matmul(
                            avp[0:65, :],
                            v_aug[b][:, kb * 65:(kb + 1) * 65],
                            st["probs"][hh][:, kb * S + qh * 512:
                                            kb * S + (qh + 1) * 512],
                            start=(kb == 0), stop=(kb == 7),
                        )

                def recip_part1(st, qh):
                    # Ln reads the denominator rows straight from PSUM: no
                    # DVE copy in the chain, so ACT never waits on Vector.
                    dcol = qh * 1024
                    for hh in range(2):
                        nc.scalar.activation(
                            rrow[64:65, dcol + hh * 512:dcol + (hh + 1) * 512],
                            st[f"av{qh}{hh}"][64:65, :], AF.Ln)
                    nc.scalar.activation(
                        denb[64:65, dcol:dcol + 1024],
                        rrow[64:65, dcol:dcol + 1024], AF.Exp, scale=-1.0)
                    sdr = dpool.tile([1, 1024], F32, tag="sdr")
                    nc.sync.dma_start(out=sdr[:], in_=denb[64:65, dcol:dcol + 1024])
                    rbc = rbcpool.tile([64, 1024], F32, tag="rbc")
                    nc.gpsimd.dma_start(out=rbc[:], in_=sdr[:].to_broadcast((64, 1024)))
                    st[f"rbc{qh}"] = rbc

                def recip_part2(st, qh):
                    rbc = st[f"rbc{qh}"]
                    avn2_t = st["avn2"]
                    nc.vector.tensor_mul(
                        avn2_t[0:64, qh * 512:(qh + 1) * 512],
                        st[f"av{qh}0"][0:64, :], rbc[0:64, 0:512])
                    scr = scrpool.tile([64, 512], BF16, tag="scr")
                    nc.vector.tensor_mul(scr[:], st[f"av{qh}1"][0:64, :],
                                         rbc[0:64, 512:1024])
                    nc.sync.dma_start(
                        out=avn2_t[64:128, qh * 512:(qh + 1) * 512], in_=scr[:])

                def outproj_half(bo, t, half):
                    bS = bo * S
                    if half == 0:
                        ustate[2 * bo]["ysb"] = ypool.tile([128, D], F32, tag="ysb", name="ysb")
                    ysb = ustate[2 * bo]["ysb"]
                    for nb in (2 * half, 2 * half + 1):
                        yps = py.tile([128, 512], F32, tag="y")
                        for pr in range(2):
                            nc.tensor.matmul(
                                yps[:],
                                ustate[2 * bo + pr]["avn2"][:, bass.ts(t, 128)],
                                wo2_sb[:, pr * D + nb * 512:pr * D + (nb + 1) * 512],
                                start=(pr == 0), stop=(pr == 1),
                            )
                        nc.vector.tensor_copy(ysb[:, bass.ts(nb, 512)], yps[:])
                    if half == 1:
                        nc.sync.dma_start(
                            out=y_d[bS + t * 128:bS + (t + 1) * 128, :], in_=ysb[:])

                for ui in range((OUT_START + 16 * B + 7) // 8 + 1):
                    for k in range(8):
                        g = ui * 8 + k
                        # qh1 recip tail of the unit whose AV ran last iter
                        if 0 <= ui - 2 < NU:
                            if k == 0:
                                recip_part1(ustate[ui - 2], 1)
                            elif k == 1:
                                recip_part2(ustate[ui - 2], 1)
                        # scores + exp
                        if ui < NU:
                            st = ustate[ui]
                            if k == 0:
                                st["u"] = units[ui]
                                pr0t = prpool.tile([128, 8 * S], BF16,
                                                   tag="prob0", name="prob0")
                                pr1t = prpool.tile([128, 8 * S], BF16,
                                                   tag="prob1", name="prob1")
                                st["probs"] = (pr0t, pr1t)
                                st["avn2"] = apool.tile(
                                    [128, S], BF16, tag=f"avn{units[ui][1]}",
                                    name=f"avn2_{ui}")
                            scores_exp(st, k)
                        # AV + qh0 recip of previous unit (av tags reuse
                        # only after the prior tile's normalize-mul)
                        if 1 <= ui <= NU:
                            st = ustate[ui - 1]
                            if k == 1:
                                av_half(st, 0, 0, 0)
                            elif k == 2:
                                av_half(st, 0, 0, 1)
                                av_half(st, 0, 1, 0)
                            elif k == 3:
                                av_half(st, 0, 1, 1)
                            elif k == 4:
                                recip_part1(st, 0)
                            elif k == 5:
                                recip_part2(st, 0)
                                av_half(st, 1, 0, 0)
                            elif k == 6:
                                av_half(st, 1, 0, 1)
                                av_half(st, 1, 1, 0)
                            elif k == 7:
                                av_half(st, 1, 1, 1)
                        # out-proj stream
                        if g >= OUT_START:
                            h = g - OUT_START
                            bo = h // 16
                            if bo < B:
                                outproj_half(bo, (h % 16) // 2, h % 2)

    try:
        nc.compile()
    finally:
        _bacc_mod.get_activation_tables = _orig_tables
    _CACHE[key] = nc
    return nc


def _host_prep(x, cos, sin, Wq, Wk, Wv, Wo):
    x = np.asarray(x, np.float32)
    xT = np.ascontiguousarray(x.reshape(T, D).T).astype(BF)
    cosT = np.asarray(cos, np.float32).T
    sinT = np.asarray(sin, np.float32).T
    cos2 = np.ascontiguousarray(np.tile(cosT, (2, 1)))          # (128, S) f32
    sin2 = np.ascontiguousarray(np.tile(sinT, (2, 1)))
    # lhsT for qshiftT = A @ qT  ->  arot = A.T (block-diag x2 over heads)
    A = np.zeros((HD, HD), np.float32)
    for d in range(32):
        A[d, d + 32] = -1.0
        A[32 + d, d] = 1.0
    arot = np.kron(np.eye(2, dtype=np.float32), A.T).astype(BF)  # (128,128)
    eye64 = np.eye(64, dtype=np.float32).astype(BF)

    Wq = np.asarray(Wq, np.float32)
    Wk = np.asarray(Wk, np.float32)
    Wv = np.asarray(Wv, np.float32)
    Wo = np.asarray(Wo, np.float32)
    in_maps = []
    for g in range(NC):
        wq_g = np.ascontiguousarray(Wq[:, g * QD:(g + 1) * QD]).astype(BF)
        wkv_g = np.ascontiguousarray(
            np.concatenate([Wk[:, g * HD:(g + 1) * HD], Wv[:, g * HD:(g + 1) * HD]], axis=1)
        ).astype(BF)
        wo_g = np.ascontiguousarray(Wo[g * QD:(g + 1) * QD, :]).astype(BF)
        in_maps.append({
            "xT": xT, "wq": wq_g, "wkv": wkv_g, "wo": wo_g,
            "cos2": cos2, "sin2": sin2, "arot": arot, "eye64": eye64,
        })
    return in_maps


def kernel(x, cos, sin, Wq, Wk, Wv, Wo):
    nc = _build()
    in_maps = _host_prep(x, cos, sin, Wq, Wk, Wv, Wo)
    res = bass_utils.run_bass_kernel_spmd(
        nc, in_maps, core_ids=list(range(NC)), trace=False,
    )
    y = np.zeros((T, D), np.float32)
    for r in res.results:
        y += np.asarray(r["y"], np.float32)
    return y.reshape(B, S, D)
